# revision 1
# baseline (speedup 1.0000x reference)
"""Trainium2 Bass kernel for nn_ExportableGENConv (GENConv message passing +
channelwise softmax aggregation + MLP with global-batch BatchNorm), sharded
across 8 NeuronCores.

Contract: kernel(**inputs) takes the FULL inputs of reference.setup_inputs()
and returns the FULL [32768, 64] float32 output.

Sharding: nodes (each with K=32 contiguous incoming edge slots) are split
across 8 cores. Per-edge source features x[src] are materialized host-side
during staging (the halo exchange), the per-edge message + per-node softmax +
MLP run on device. Global BatchNorm statistics are combined on host between
two NEFF launches (in-kernel collectives fail to load under this runtime):
phase 1 produces h1 = (aggregated+x) @ W1 plus per-core sum/sumsq, phase 2
applies the batch-norm affine + ReLU + W2.

Math (per node i, channel h, over valid slots k):
  t = x[src] + ea @ W_edge            (invalid slots: staged x = -1e9 -> t << 0)
  reference: m = relu(t) + 1e-7; softmax over k of m; res = sum m*alpha.
  With r = relu(t):  res = (sum_k r*e^r) / (sum_k e^r + 1e-16) + 1e-7
  Device: E = exp(t); P = max(E, 1) (= e^r; invalid slots contribute exactly
  1.0, removed via a host-staged per-node count); Pm = max(t, 0)*E (= r*e^r;
  invalid slots contribute exactly 0).

Device layout ("k-layout"): node-tile = 32 nodes = 1024 edge slots laid out as
partition p = (node%32)*4 + (k%4), free = (b=k//4 in [0,8), h). The softmax
reduction is a PE matmul with a block-diagonal ones stationary over partition
blocks of 4 (stage 1; col-offset-stacked over 4 node-tiles to fill all 128
PSUM partitions) + a DVE reduce over b (stage 2, innermost-strided AP).
"""

import numpy as np
from contextlib import ExitStack

import concourse.bass as bass
import concourse.tile as tile
from concourse import mybir
from concourse.bass_utils import run_bass_kernel_spmd

# ---------------------------------------------------------------- constants
N, K, H, ED = 32768, 32, 64, 32
E = N * K
NCORES = 8
NPC = N // NCORES            # nodes per core = 4096
NT = NPC // 32               # node-tiles per core = 128
NEG_BIG = -1.0e9

_compiled = {}


# ------------------------------------------------------- multi-wait legalizer
def _legalize_multiwaits(nc):
    """This walrus build accepts only ONE sync wait per instruction; move the
    excess onto injected same-engine drain carriers placed immediately before
    the instruction (semantics-preserving: the engine stalls there instead)."""
    n_injected = 0
    for fn in nc.m.functions:
        for blk in fn.blocks:
            bb = blk if hasattr(blk, "instructions") else blk.bb
            insts = list(bb.instructions)
            out = []
            for inst in insts:
                si = inst.sync_info
                if si is not None and si.on_wait and len(si.on_wait) > 1:
                    waits = list(si.on_wait)
                    for w in waits[:-1]:
                        nop = mybir.InstDrain(
                            name=f"waitfix-{nc.next_id()}", ins=[], outs=[]
                        )
                        nop.engine = inst.engine
                        nop.sync_info = mybir.SyncInfo(on_wait=[w], on_update=[])
                        nc.register_instruction(nop, overwrite=True)
                        out.append(nop)
                        n_injected += 1
                    inst.sync_info = mybir.SyncInfo(
                        on_wait=[waits[-1]], on_update=list(si.on_update or [])
                    )
                out.append(inst)
            if len(out) != len(insts):
                bb.instructions = out
    return n_injected


# ------------------------------------------------------------ phase-1 kernel
def _build_phase1(ntiles=NT):
    fp32 = mybir.dt.float32
    Act = mybir.ActivationFunctionType
    Alu = mybir.AluOpType
    npc = ntiles * 32
    ngrp = ntiles // 4
    nc = bass.Bass()

    xs_d = nc.declare_dram_parameter("xs", [128, ntiles * 512], fp32, isOutput=False)
    ea4_d = nc.declare_dram_parameter("ea4", [128, ntiles * 256], fp32, isOutput=False)
    wbd_d = nc.declare_dram_parameter("wbd", [128, 256], fp32, isOutput=False)
    bd_d = nc.declare_dram_parameter("bd", [128, 32], fp32, isOutput=False)
    id_d = nc.declare_dram_parameter("ident", [128, 128], fp32, isOutput=False)
    corr_d = nc.declare_dram_parameter("corr", [128, ngrp * 64], fp32, isOutput=False)
    xres_d = nc.declare_dram_parameter("xres", [128, ngrp * 64], fp32, isOutput=False)
    w1_d = nc.declare_dram_parameter("w1", [64, 128], fp32, isOutput=False)
    h1_d = nc.declare_dram_parameter("h1", [128, npc], fp32, isOutput=True)
    st_d = nc.declare_dram_parameter("stats", [128, 2], fp32, isOutput=True)

    with tile.TileContext(nc) as tc, ExitStack() as ctx:
        const = ctx.enter_context(tc.tile_pool(name="const", bufs=1))
        sb = ctx.enter_context(tc.tile_pool(name="sb", bufs=1))
        xs_p = ctx.enter_context(tc.tile_pool(name="xs", bufs=3))
        ea_p = ctx.enter_context(tc.tile_pool(name="ea", bufs=3))
        e_p = ctx.enter_context(tc.tile_pool(name="ee", bufs=3))
        pp_p = ctx.enter_context(tc.tile_pool(name="pp", bufs=2))
        pm_p = ctx.enter_context(tc.tile_pool(name="pm", bufs=2))
        ps_t = ctx.enter_context(tc.tile_pool(name="ps_t", bufs=3, space="PSUM"))
        ps_s = ctx.enter_context(tc.tile_pool(name="ps_s", bufs=2, space="PSUM"))
        ps_u = ctx.enter_context(tc.tile_pool(name="ps_u", bufs=2, space="PSUM"))

        wbd_t = const.tile([128, 256], fp32)
        nc.sync.dma_start(wbd_t[:], wbd_d[:])
        bd_t = const.tile([128, 32], fp32)
        nc.sync.dma_start(bd_t[:], bd_d[:])
        id_t = const.tile([128, 128], fp32)
        nc.sync.dma_start(id_t[:], id_d[:])
        corr_t = const.tile([128, ngrp * 64], fp32)
        nc.sync.dma_start(corr_t[:], corr_d[:])
        xres_t = const.tile([128, ngrp * 64], fp32)
        nc.sync.dma_start(xres_t[:], xres_d[:])
        w1_t = const.tile([64, 128], fp32)
        nc.sync.dma_start(w1_t[:], w1_d[:])

        S2_all = sb.tile([128, ngrp * 64], fp32)
        T2_all = sb.tile([128, ngrp * 64], fp32)

        # prologue: make PE observe const DMA sems via tiny touch matmuls
        pro_ps = ps_u.tile([128, 512], fp32, tag="ps_u")
        nc.tensor.matmul(pro_ps[0:8, 0:8], id_t[:, 0:8], id_t[:, 0:8],
                         start=True, stop=True, skip_group_check=True)
        nc.tensor.matmul(pro_ps[0:8, 0:8], wbd_t[:, 0:8], wbd_t[:, 0:8],
                         start=True, stop=True, skip_group_check=True)
        nc.tensor.matmul(pro_ps[0:8, 0:8], bd_t[:, 0:8], bd_t[:, 0:8],
                         start=True, stop=True, skip_group_check=True)

        # ---- edge phase
        s1_pair = [None, None]
        for T in range(ntiles):
            G, c = divmod(T, 4)
            xs_t = xs_p.tile([128, 512], fp32, tag="xs")
            nc.sync.dma_start(xs_t[:], xs_d[:, T * 512:(T + 1) * 512])
            ea_t = ea_p.tile([128, 2, 128], fp32, tag="ea")
            nc.sync.dma_start(
                ea_t[:],
                ea4_d[:, T * 256:(T + 1) * 256].rearrange("p (g e) -> p g e", g=2))

            t_ps = ps_t.tile([128, 512], fp32, tag="ps_t")
            nc.tensor.matmul(t_ps[:], id_t[:], xs_t[:], start=True, stop=False)
            for g in range(2):
                nc.tensor.matmul(t_ps[:, 256 * g:256 * (g + 1)], ea_t[:, g, :],
                                 wbd_t[:], start=False, stop=(g == 1))

            E_t = e_p.tile([128, 512], fp32, tag="ee")
            nc.scalar.activation(E_t[:], t_ps[:], Act.Exp)
            P_t = pp_p.tile([128, 512], fp32, tag="pp")
            nc.vector.tensor_scalar_max(P_t[:], E_t[:], 1.0)
            Pm_t = pm_p.tile([128, 512], fp32, tag="pm")
            nc.vector.scalar_tensor_tensor(
                Pm_t[:], t_ps[:], 0.0, E_t[:], op0=Alu.max, op1=Alu.mult)

            if c == 0:
                s1_pair = [ps_s.tile([128, 512], fp32, tag="ps_s", name=f"S1_{T}"),
                           ps_u.tile([128, 512], fp32, tag="ps_u", name=f"T1_{T}")]
            S1_ps, T1_ps = s1_pair
            nc.tensor.matmul(S1_ps[32 * c:32 * c + 32, :], bd_t[:], P_t[:],
                             start=True, stop=True, tile_position=(0, 32 * c),
                             skip_group_check=True)
            nc.tensor.matmul(T1_ps[32 * c:32 * c + 32, :], bd_t[:], Pm_t[:],
                             start=True, stop=True, tile_position=(0, 32 * c),
                             skip_group_check=True)

            if c == 3:
                nc.vector.tensor_reduce(
                    S2_all[:, G * 64:(G + 1) * 64],
                    S1_ps[:].rearrange("p (b h) -> p h b", h=H),
                    axis=mybir.AxisListType.X, op=Alu.add)
                nc.vector.tensor_reduce(
                    T2_all[:, G * 64:(G + 1) * 64],
                    T1_ps[:].rearrange("p (b h) -> p h b", h=H),
                    axis=mybir.AxisListType.X, op=Alu.add)

        # ---- per-node combine: res = T2/(S2 - corr) + 1e-7 + xres
        # (corr is staged as ninv - 1e-16 so the subtraction also adds the
        # softmax eps to the denominator.)
        den = sb.tile([128, ngrp * 64], fp32)
        nc.vector.tensor_tensor(out=den[:], in0=S2_all[:], in1=corr_t[:],
                                op=Alu.subtract)
        rec = sb.tile([128, ngrp * 64], fp32)
        nc.vector.reciprocal(rec[:], den[:])
        res = sb.tile([128, ngrp * 64], fp32)
        nc.vector.tensor_tensor(out=res[:], in0=T2_all[:], in1=rec[:], op=Alu.mult)
        nc.vector.scalar_tensor_tensor(
            res[:], res[:], 1e-7, xres_t[:], op0=Alu.add, op1=Alu.add)

        # ---- transpose res blocks to feature-major outT [64, npc]
        outT = sb.tile([64, npc], fp32)
        for G2 in range(ngrp):
            tr_ps = ps_t.tile([128, 512], fp32, tag="ps_t")
            nc.tensor.transpose(tr_ps[0:64, 0:128],
                                res[:, G2 * 64:(G2 + 1) * 64], id_t[:])
            nc.scalar.copy(outT[:, G2 * 128:(G2 + 1) * 128], tr_ps[0:64, 0:128])

        # ---- h1 = W1.T @ outT  (feature-major), write to DRAM + stats
        h1 = sb.tile([128, npc], fp32)
        for j in range(npc // 512):
            h1_ps = ps_s.tile([128, 512], fp32, tag="ps_s")
            nc.tensor.matmul(h1_ps[:], w1_t[:], outT[:, j * 512:(j + 1) * 512],
                             start=True, stop=True, skip_group_check=True)
            nc.scalar.copy(h1[:, j * 512:(j + 1) * 512], h1_ps[:])
        nc.sync.dma_start(h1_d[:], h1[:])

        s1 = sb.tile([128, 1], fp32)
        nc.vector.tensor_reduce(s1[:], h1[:], axis=mybir.AxisListType.X, op=Alu.add)
        sqscr = sb.tile([128, npc], fp32)
        s2 = sb.tile([128, 1], fp32)
        nc.vector.scalar_tensor_tensor(
            sqscr[:], h1[:], 0.0, h1[:], op0=Alu.add, op1=Alu.mult,
            accum_out=s2[:])
        stats = sb.tile([128, 2], fp32)
        nc.vector.tensor_copy(stats[:, 0:1], s1[:])
        nc.vector.tensor_copy(stats[:, 1:2], s2[:])
        nc.scalar.dma_start(st_d[:], stats[:])

    _legalize_multiwaits(nc)
    return nc


# ------------------------------------------------------------ phase-2 kernel
def _build_phase2(ntiles=NT):
    fp32 = mybir.dt.float32
    Act = mybir.ActivationFunctionType
    npc = ntiles * 32
    nc = bass.Bass()

    h1_d = nc.declare_dram_parameter("h1", [128, npc], fp32, isOutput=False)
    ss_d = nc.declare_dram_parameter("ss", [128, 2], fp32, isOutput=False)
    w2_d = nc.declare_dram_parameter("w2", [128, 64], fp32, isOutput=False)
    out_d = nc.declare_dram_parameter("out", [64, npc], fp32, isOutput=True)

    with tile.TileContext(nc) as tc, ExitStack() as ctx:
        const = ctx.enter_context(tc.tile_pool(name="const", bufs=1))
        sb = ctx.enter_context(tc.tile_pool(name="sb", bufs=1))
        osl_p = ctx.enter_context(tc.tile_pool(name="osl", bufs=3))
        ps = ctx.enter_context(tc.tile_pool(name="ps", bufs=3, space="PSUM"))

        ss_t = const.tile([128, 2], fp32)
        nc.sync.dma_start(ss_t[:], ss_d[:])
        w2_t = const.tile([128, 64], fp32)
        nc.sync.dma_start(w2_t[:], w2_d[:])
        h1 = sb.tile([128, npc], fp32)
        nc.sync.dma_start(h1[:], h1_d[:])

        h2 = sb.tile([128, npc], fp32)
        nc.scalar.activation(h2[:], h1[:], Act.Relu, bias=ss_t[:, 1:2],
                             scale=ss_t[:, 0:1])
        for j in range(npc // 512):
            o_ps = ps.tile([128, 512], fp32, tag="ps")
            nc.tensor.matmul(o_ps[0:64, :], w2_t[:], h2[:, j * 512:(j + 1) * 512],
                             start=True, stop=True, skip_group_check=True)
            oslice = osl_p.tile([64, 512], fp32, tag="osl")
            nc.scalar.copy(oslice[:], o_ps[0:64, :])
            nc.scalar.dma_start(out_d[:, j * 512:(j + 1) * 512], oslice[:])

    _legalize_multiwaits(nc)
    return nc


# -------------------------------------------------------------- host staging
def _stage_core(x_c, xs_slot_c, ea_slot_c, ninv_c, ntiles=NT):
    """xs_slot_c: [npc, K, H] f32 (x[src], invalid slots = NEG_BIG)
    ea_slot_c: [npc, K, ED] f32;  ninv_c: [npc] f32."""
    a = xs_slot_c.reshape(ntiles, 32, 8, 4, H)          # [T, m, b, j, h]
    xs_dev = np.ascontiguousarray(
        a.transpose(1, 3, 0, 2, 4)).reshape(128, ntiles * 512)

    b = ea_slot_c.reshape(ntiles, 32, 8, 4, ED)         # [T, m, b, j, d]
    ea4 = np.ascontiguousarray(
        b.transpose(2, 4, 0, 1, 3)                      # [b, d, T, m, j]
        .reshape(2, 4, ED, ntiles, 128)                 # [g, r, d, T, e']
        .transpose(1, 2, 3, 0, 4)                       # [r, d, T, g, e']
    ).reshape(128, ntiles * 256)

    # node n = 128*G + p'  (p' = 32*(T%4) + node%32)
    corr = (ninv_c.astype(np.float32) - 1e-16)[:, None] * np.ones((1, H), np.float32)
    corr_dev = np.ascontiguousarray(
        corr.reshape(ntiles // 4, 128, H).transpose(1, 0, 2)).reshape(128, -1)
    xres_dev = np.ascontiguousarray(
        x_c.reshape(ntiles // 4, 128, H).transpose(1, 0, 2)).reshape(128, -1)
    return xs_dev, ea4, corr_dev, xres_dev


def _consts(W_edge):
    Wbd = np.zeros((128, 256), np.float32)
    for r in range(4):
        Wbd[32 * r:32 * r + 32, 64 * r:64 * r + 64] = W_edge
    BD = np.zeros((128, 32), np.float32)
    for m in range(32):
        BD[4 * m:4 * m + 4, m] = 1.0
    ident = np.eye(128, dtype=np.float32)
    return Wbd, BD, ident


def kernel(x, edge_index, edge_attr, nbr, W_edge, W1, gamma, beta, W2):
    x = np.ascontiguousarray(np.asarray(x, dtype=np.float32))
    edge_index = np.asarray(edge_index)
    edge_attr = np.ascontiguousarray(np.asarray(edge_attr, dtype=np.float32))
    nbr = np.asarray(nbr)
    W_edge = np.ascontiguousarray(np.asarray(W_edge, dtype=np.float32))
    W1 = np.ascontiguousarray(np.asarray(W1, dtype=np.float32))
    gamma = np.asarray(gamma, dtype=np.float32)
    beta = np.asarray(beta, dtype=np.float32)
    W2 = np.ascontiguousarray(np.asarray(W2, dtype=np.float32))

    src = np.asarray(edge_index[0], dtype=np.int64)
    valid = nbr >= 0                                    # [N, K]
    expect = np.arange(E, dtype=np.int64).reshape(N, K)
    assert np.array_equal(np.where(valid, nbr, expect), expect), \
        "kernel assumes nbr[i,k] == i*K+k on valid slots"

    src_slot = src.reshape(N, K)
    xs_slot = x[src_slot]                               # host halo: [N, K, H]
    xs_slot[~valid] = NEG_BIG
    ninv = (~valid).sum(axis=1).astype(np.float32)      # [N]
    ea_slot = edge_attr.reshape(N, K, ED)

    Wbd, BD, ident = _consts(W_edge)

    if "p1" not in _compiled:
        _compiled["p1"] = _build_phase1(NT)
        _compiled["p2"] = _build_phase2(NT)

    in_maps = []
    for core in range(NCORES):
        sl = slice(core * NPC, (core + 1) * NPC)
        xs_dev, ea4, corr_dev, xres_dev = _stage_core(
            x[sl], xs_slot[sl], ea_slot[sl], ninv[sl])
        in_maps.append({
            "xs": xs_dev, "ea4": ea4, "wbd": Wbd, "bd": BD, "ident": ident,
            "corr": corr_dev, "xres": xres_dev, "w1": W1,
        })

    res1 = run_bass_kernel_spmd(_compiled["p1"], in_maps,
                                core_ids=list(range(NCORES)))

    # host: combine BN stats (tiny 128-vector arithmetic), build scale/shift
    s1 = np.zeros(2 * H, np.float64)
    s2 = np.zeros(2 * H, np.float64)
    for core in range(NCORES):
        st = res1.results[core]["stats"].astype(np.float64)
        s1 += st[:, 0]
        s2 += st[:, 1]
    mean = (s1 / N).astype(np.float32)
    var = (s2 / N).astype(np.float32) - mean * mean
    scale = gamma / np.sqrt(var + 1e-5)
    shift = beta - mean * scale
    ss = np.stack([scale, shift], axis=1).astype(np.float32)  # [128, 2]

    in_maps2 = [{"h1": res1.results[core]["h1"], "ss": ss, "w2": W2}
                for core in range(NCORES)]
    res2 = run_bass_kernel_spmd(_compiled["p2"], in_maps2,
                                core_ids=list(range(NCORES)))

    out = np.empty((N, H), np.float32)
    for core in range(NCORES):
        out[core * NPC:(core + 1) * NPC] = res2.results[core]["out"].T
    return out



# revision 2
# speedup vs baseline: 2.3454x; 2.3454x over previous
"""Trainium2 Bass kernel for nn_ExportableGENConv (GENConv message passing +
channelwise softmax aggregation + MLP with global-batch BatchNorm), sharded
across 8 NeuronCores.

Contract: kernel(**inputs) takes the FULL inputs of reference.setup_inputs()
and returns the FULL [32768, 64] float32 output.

Sharding: nodes (each with K=32 contiguous incoming edge slots) are split
across 8 cores. Per-edge source features x[src] are materialized host-side
during staging (the halo exchange) in bf16, the per-edge message + per-node
softmax + MLP run on device. Global BatchNorm statistics are combined on host
between two NEFF launches (in-kernel collectives fail to load under this
runtime): phase 1 produces h1 = (aggregated+x) @ W1 plus per-core sum/sumsq,
phase 2 applies the batch-norm affine + ReLU + W2.

Math (per node i, channel h, over valid slots k):
  t = x[src] + ea @ W_edge            (invalid slots: staged x = -1e9 -> t << 0)
  reference: m = relu(t) + 1e-7; softmax over k of m; res = sum m*alpha.
  With r = relu(t):  res = (sum_k r*e^r) / (sum_k e^r + 1e-16) + 1e-7
  Device: r = relu(t) (fp16); P = exp(r) (bf16; invalid slots contribute
  exactly 1.0, removed via a host-staged per-node count); Pm = r*P (bf16).

Device layout ("k-layout"): node-tile = 32 nodes = 1024 edge slots laid out as
partition p = (node%32)*4 + (k%4), free = (b=k//4 in [0,8), h). Tiles are
processed in chunks of 2 (one [128,1024] PSUM pair-bank per chunk) so the
scalar/vector elementwise passes amortize their fixed access latency. The
softmax reduction is a PE matmul with a block-diagonal ones stationary over
partition blocks of 4 (stage 1; col-offset-stacked over 4 node-tiles to fill
all 128 PSUM partitions) + a DVE reduce over b (stage 2, innermost-strided
AP). The relu pass alternates scalar/vector engines (2:1) to balance load.
"""

import numpy as np
from contextlib import ExitStack

import concourse.bass as bass
import concourse.tile as tile
from concourse import mybir
from concourse.bass_utils import run_bass_kernel_spmd

try:
    from ml_dtypes import bfloat16 as np_bf16, float16 as np_fp16
except ImportError:  # ml_dtypes ships with jax; fall back just in case
    import jax.numpy as _jnp

    np_bf16 = _jnp.bfloat16
    np_fp16 = np.float16

# ---------------------------------------------------------------- constants
N, K, H, ED = 32768, 32, 64, 32
E = N * K
NCORES = 8
NPC = N // NCORES            # nodes per core = 4096
NT = NPC // 32               # node-tiles per core = 128
NCH = NT // 2                # 2-tile chunks per core = 64
NEG_BIG = -1.0e9

_compiled = {}


# ------------------------------------------------------- multi-wait legalizer
def _legalize_multiwaits(nc):
    """This walrus build accepts only ONE sync wait per instruction; move the
    excess onto injected same-engine drain carriers placed immediately before
    the instruction (semantics-preserving: the engine stalls there instead)."""
    n_injected = 0
    for fn in nc.m.functions:
        for blk in fn.blocks:
            bb = blk if hasattr(blk, "instructions") else blk.bb
            insts = list(bb.instructions)
            out = []
            for inst in insts:
                si = inst.sync_info
                if si is not None and si.on_wait and len(si.on_wait) > 1:
                    waits = list(si.on_wait)
                    for w in waits[:-1]:
                        nop = mybir.InstDrain(
                            name=f"waitfix-{nc.next_id()}", ins=[], outs=[]
                        )
                        nop.engine = inst.engine
                        nop.sync_info = mybir.SyncInfo(on_wait=[w], on_update=[])
                        nc.register_instruction(nop, overwrite=True)
                        out.append(nop)
                        n_injected += 1
                    inst.sync_info = mybir.SyncInfo(
                        on_wait=[waits[-1]], on_update=list(si.on_update or [])
                    )
                out.append(inst)
            if len(out) != len(insts):
                bb.instructions = out
    return n_injected


# ------------------------------------------------------------ phase-1 kernel
def _build_phase1(ntiles=NT):
    fp32 = mybir.dt.float32
    bf16 = mybir.dt.bfloat16
    fp16 = mybir.dt.float16
    Act = mybir.ActivationFunctionType
    Alu = mybir.AluOpType
    npc = ntiles * 32
    ngrp = ntiles // 4
    nchunk = ntiles // 2
    nc = bass.Bass()

    xae_d = nc.declare_dram_parameter("xae", [128, nchunk * 1536], bf16,
                                      isOutput=False)
    wbd_d = nc.declare_dram_parameter("wbd", [128, 256], bf16, isOutput=False)
    bd_d = nc.declare_dram_parameter("bd", [128, 32], bf16, isOutput=False)
    id16_d = nc.declare_dram_parameter("id16", [128, 128], bf16, isOutput=False)
    id32_d = nc.declare_dram_parameter("id32", [128, 128], fp32, isOutput=False)
    corr_d = nc.declare_dram_parameter("corr", [128, ngrp * 64], fp32, isOutput=False)
    xres_d = nc.declare_dram_parameter("xres", [128, ngrp * 64], fp32, isOutput=False)
    w1_d = nc.declare_dram_parameter("w1", [64, 128], fp32, isOutput=False)
    h1_d = nc.declare_dram_parameter("h1", [128, npc], fp32, isOutput=True)
    st_d = nc.declare_dram_parameter("stats", [128, 2], fp32, isOutput=True)

    with tile.TileContext(nc) as tc, ExitStack() as ctx:
        const = ctx.enter_context(tc.tile_pool(name="const", bufs=1))
        sb = ctx.enter_context(tc.tile_pool(name="sb", bufs=1))
        xae_p = ctx.enter_context(tc.tile_pool(name="xae", bufs=3))
        r_p = ctx.enter_context(tc.tile_pool(name="rr", bufs=3))
        pp_p = ctx.enter_context(tc.tile_pool(name="pp", bufs=2))
        pm_p = ctx.enter_context(tc.tile_pool(name="pm", bufs=2))
        ps_t = ctx.enter_context(tc.tile_pool(name="ps_t", bufs=2, space="PSUM"))
        ps_s = ctx.enter_context(tc.tile_pool(name="ps_s", bufs=2, space="PSUM"))
        ps_u = ctx.enter_context(tc.tile_pool(name="ps_u", bufs=2, space="PSUM"))

        wbd_t = const.tile([128, 256], bf16)
        nc.sync.dma_start(wbd_t[:], wbd_d[:])
        bd_t = const.tile([128, 32], bf16)
        nc.sync.dma_start(bd_t[:], bd_d[:])
        id16_t = const.tile([128, 128], bf16)
        nc.sync.dma_start(id16_t[:], id16_d[:])
        id32_t = const.tile([128, 128], fp32)
        nc.sync.dma_start(id32_t[:], id32_d[:])
        corr_t = const.tile([128, ngrp * 64], fp32)
        nc.sync.dma_start(corr_t[:], corr_d[:])
        xres_t = const.tile([128, ngrp * 64], fp32)
        nc.sync.dma_start(xres_t[:], xres_d[:])
        w1_t = const.tile([64, 128], fp32)
        nc.sync.dma_start(w1_t[:], w1_d[:])

        S2_all = sb.tile([128, ngrp * 64], fp32)
        T2_all = sb.tile([128, ngrp * 64], fp32)

        # prologue: make PE observe const DMA sems via tiny touch matmuls
        pro_ps = ps_u.tile([128, 512], fp32, tag="ps_u")
        nc.tensor.matmul(pro_ps[0:8, 0:8], id16_t[:, 0:8], id16_t[:, 0:8],
                         start=True, stop=True, skip_group_check=True)
        nc.tensor.matmul(pro_ps[0:8, 0:8], wbd_t[:, 0:8], wbd_t[:, 0:8],
                         start=True, stop=True, skip_group_check=True)
        nc.tensor.matmul(pro_ps[0:8, 0:8], bd_t[:, 0:8], bd_t[:, 0:8],
                         start=True, stop=True, skip_group_check=True)
        nc.tensor.matmul(pro_ps[0:8, 0:8], id32_t[:, 0:8], id32_t[:, 0:8],
                         start=True, stop=True, skip_group_check=True)
        nc.tensor.matmul(pro_ps[0:8, 0:8], w1_t[:, 0:8], w1_t[:, 0:8],
                         start=True, stop=True, skip_group_check=True)

        # ---- edge phase (chunks of 2 node-tiles = 2048 edges)
        s1_pair = [None, None]
        for j in range(nchunk):
            xae_t = xae_p.tile([128, 1536], bf16, tag="xae")
            nc.sync.dma_start(xae_t[:], xae_d[:, j * 1536:(j + 1) * 1536])

            t_ps = ps_t.tile([128, 1024], fp32, tag="ps_t")
            for i in range(2):
                nc.tensor.matmul(t_ps[:, 512 * i:512 * (i + 1)], id16_t[:],
                                 xae_t[:, 512 * i:512 * (i + 1)],
                                 start=True, stop=False)
                for g in range(2):
                    nc.tensor.matmul(
                        t_ps[:, 512 * i + 256 * g:512 * i + 256 * (g + 1)],
                        xae_t[:, 1024 + 256 * i + 128 * g:
                              1024 + 256 * i + 128 * (g + 1)],
                        wbd_t[:], start=False, stop=(g == 1))

            # r = relu(t): 2 of 3 chunks on scalar, 1 of 3 on vector
            r_t = r_p.tile([128, 1024], fp16, tag="rr")
            if j % 3 == 2:
                nc.vector.tensor_scalar_max(r_t[:], t_ps[:], 0.0)
            else:
                nc.scalar.activation(r_t[:], t_ps[:], Act.Relu)
            # P = exp(r) (scalar); Pm = r * P (vector, 2x bf16 mode)
            P_t = pp_p.tile([128, 1024], bf16, tag="pp")
            nc.scalar.activation(P_t[:], r_t[:], Act.Exp)
            Pm_t = pm_p.tile([128, 1024], bf16, tag="pm")
            nc.vector.tensor_tensor(out=Pm_t[:], in0=r_t[:], in1=P_t[:],
                                    op=Alu.mult)

            for i in range(2):
                T = 2 * j + i
                c = T % 4
                if c == 0:
                    s1_pair = [
                        ps_s.tile([128, 512], fp32, tag="ps_s", name=f"S1_{T}"),
                        ps_u.tile([128, 512], fp32, tag="ps_u", name=f"T1_{T}"),
                    ]
                S1_ps, T1_ps = s1_pair
                nc.tensor.matmul(S1_ps[32 * c:32 * c + 32, :], bd_t[:],
                                 P_t[:, 512 * i:512 * (i + 1)],
                                 start=True, stop=True, tile_position=(0, 32 * c),
                                 skip_group_check=True)
                nc.tensor.matmul(T1_ps[32 * c:32 * c + 32, :], bd_t[:],
                                 Pm_t[:, 512 * i:512 * (i + 1)],
                                 start=True, stop=True, tile_position=(0, 32 * c),
                                 skip_group_check=True)

                if c == 3:
                    G = T // 4
                    nc.vector.tensor_reduce(
                        S2_all[:, G * 64:(G + 1) * 64],
                        S1_ps[:].rearrange("p (b h) -> p h b", h=H),
                        axis=mybir.AxisListType.X, op=Alu.add)
                    nc.vector.tensor_reduce(
                        T2_all[:, G * 64:(G + 1) * 64],
                        T1_ps[:].rearrange("p (b h) -> p h b", h=H),
                        axis=mybir.AxisListType.X, op=Alu.add)

        # ---- per-node combine: res = T2/(S2 - corr) + 1e-7 + xres
        # (corr is staged as ninv - 1e-16 so the subtraction also adds the
        # softmax eps to the denominator.)
        den = sb.tile([128, ngrp * 64], fp32)
        nc.vector.tensor_tensor(out=den[:], in0=S2_all[:], in1=corr_t[:],
                                op=Alu.subtract)
        rec = sb.tile([128, ngrp * 64], fp32)
        nc.vector.reciprocal(rec[:], den[:])
        res = sb.tile([128, ngrp * 64], fp32)
        nc.vector.tensor_tensor(out=res[:], in0=T2_all[:], in1=rec[:], op=Alu.mult)
        nc.vector.scalar_tensor_tensor(
            res[:], res[:], 1e-7, xres_t[:], op0=Alu.add, op1=Alu.add)

        # ---- transpose res blocks to feature-major outT [64, npc]
        outT = sb.tile([64, npc], fp32)
        for G2 in range(ngrp):
            tr_ps = ps_t.tile([128, 1024], fp32, tag="ps_t")
            nc.tensor.transpose(tr_ps[0:64, 0:128],
                                res[:, G2 * 64:(G2 + 1) * 64], id32_t[:])
            nc.scalar.copy(outT[:, G2 * 128:(G2 + 1) * 128], tr_ps[0:64, 0:128])

        # ---- h1 = W1.T @ outT  (feature-major), write to DRAM + stats
        h1 = sb.tile([128, npc], fp32)
        for j in range(npc // 512):
            h1_ps = ps_s.tile([128, 512], fp32, tag="ps_s")
            nc.tensor.matmul(h1_ps[:], w1_t[:], outT[:, j * 512:(j + 1) * 512],
                             start=True, stop=True, skip_group_check=True)
            nc.scalar.copy(h1[:, j * 512:(j + 1) * 512], h1_ps[:])
        nc.sync.dma_start(h1_d[:], h1[:])

        s1 = sb.tile([128, 1], fp32)
        nc.vector.tensor_reduce(s1[:], h1[:], axis=mybir.AxisListType.X, op=Alu.add)
        sqscr = sb.tile([128, npc], fp32)
        s2 = sb.tile([128, 1], fp32)
        nc.vector.scalar_tensor_tensor(
            sqscr[:], h1[:], 0.0, h1[:], op0=Alu.add, op1=Alu.mult,
            accum_out=s2[:])
        stats = sb.tile([128, 2], fp32)
        nc.vector.tensor_copy(stats[:, 0:1], s1[:])
        nc.vector.tensor_copy(stats[:, 1:2], s2[:])
        nc.scalar.dma_start(st_d[:], stats[:])

    _legalize_multiwaits(nc)
    return nc


# ------------------------------------------------------------ phase-2 kernel
def _build_phase2(ntiles=NT):
    fp32 = mybir.dt.float32
    Act = mybir.ActivationFunctionType
    npc = ntiles * 32
    nc = bass.Bass()

    h1_d = nc.declare_dram_parameter("h1", [128, npc], fp32, isOutput=False)
    ss_d = nc.declare_dram_parameter("ss", [128, 2], fp32, isOutput=False)
    w2_d = nc.declare_dram_parameter("w2", [128, 64], fp32, isOutput=False)
    out_d = nc.declare_dram_parameter("out", [64, npc], fp32, isOutput=True)

    with tile.TileContext(nc) as tc, ExitStack() as ctx:
        const = ctx.enter_context(tc.tile_pool(name="const", bufs=1))
        sb = ctx.enter_context(tc.tile_pool(name="sb", bufs=1))
        osl_p = ctx.enter_context(tc.tile_pool(name="osl", bufs=3))
        ps = ctx.enter_context(tc.tile_pool(name="ps", bufs=3, space="PSUM"))

        ss_t = const.tile([128, 2], fp32)
        nc.sync.dma_start(ss_t[:], ss_d[:])
        w2_t = const.tile([128, 64], fp32)
        nc.sync.dma_start(w2_t[:], w2_d[:])
        h1 = sb.tile([128, npc], fp32)
        nc.sync.dma_start(h1[:], h1_d[:])

        h2 = sb.tile([128, npc], fp32)
        nc.scalar.activation(h2[:], h1[:], Act.Relu, bias=ss_t[:, 1:2],
                             scale=ss_t[:, 0:1])
        for j in range(npc // 512):
            o_ps = ps.tile([128, 512], fp32, tag="ps")
            nc.tensor.matmul(o_ps[0:64, :], w2_t[:], h2[:, j * 512:(j + 1) * 512],
                             start=True, stop=True, skip_group_check=True)
            oslice = osl_p.tile([64, 512], fp32, tag="osl")
            nc.scalar.copy(oslice[:], o_ps[0:64, :])
            nc.scalar.dma_start(out_d[:, j * 512:(j + 1) * 512], oslice[:])

    _legalize_multiwaits(nc)
    return nc


# -------------------------------------------------------------- host staging
def _stage_core(x_c, xs_slot_c, ea_slot_c, ninv_c, ntiles=NT):
    """xs_slot_c: [npc, K, H] f32 (x[src], invalid slots = NEG_BIG)
    ea_slot_c: [npc, K, ED] f32;  ninv_c: [npc] f32.

    Returns (xae bf16 [128, nchunk*1536], corr f32, xres f32)."""
    a = xs_slot_c.reshape(ntiles, 32, 8, 4, H)          # [T, m, b, j, h]
    xs_dev = np.ascontiguousarray(
        a.transpose(1, 3, 0, 2, 4)).reshape(128, ntiles * 512)

    b = ea_slot_c.reshape(ntiles, 32, 8, 4, ED)         # [T, m, b, j, d]
    ea4 = np.ascontiguousarray(
        b.transpose(2, 4, 0, 1, 3)                      # [b, d, T, m, j]
        .reshape(2, 4, ED, ntiles, 128)                 # [g, r, d, T, e']
        .transpose(1, 2, 3, 0, 4)                       # [r, d, T, g, e']
    ).reshape(128, ntiles * 256)

    # fuse xs + ea into one per-chunk DMA block:
    # chunk j: [xs(2j) 512 | xs(2j+1) 512 | ea(2j) 256 | ea(2j+1) 256]
    nch = ntiles // 2
    xs3 = xs_dev.reshape(128, nch, 1024)
    ea3 = ea4.reshape(128, nch, 512)
    xae = np.concatenate([xs3, ea3], axis=2).astype(np_bf16)
    xae = np.ascontiguousarray(xae).reshape(128, nch * 1536)

    # node n = 128*G + p'  (p' = 32*(T%4) + node%32)
    corr = (ninv_c.astype(np.float32) - 1e-16)[:, None] * np.ones((1, H), np.float32)
    corr_dev = np.ascontiguousarray(
        corr.reshape(ntiles // 4, 128, H).transpose(1, 0, 2)).reshape(128, -1)
    xres_dev = np.ascontiguousarray(
        x_c.reshape(ntiles // 4, 128, H).transpose(1, 0, 2)).reshape(128, -1)
    return xae, corr_dev, xres_dev


def _consts(W_edge):
    Wbd = np.zeros((128, 256), np.float32)
    for r in range(4):
        Wbd[32 * r:32 * r + 32, 64 * r:64 * r + 64] = W_edge
    BD = np.zeros((128, 32), np.float32)
    for m in range(32):
        BD[4 * m:4 * m + 4, m] = 1.0
    ident = np.eye(128, dtype=np.float32)
    return Wbd, BD, ident


def build_in_maps(x, edge_index, edge_attr, nbr, W_edge, W1):
    """Stage the full inputs into per-core phase-1 input maps."""
    x = np.ascontiguousarray(np.asarray(x, dtype=np.float32))
    edge_attr = np.ascontiguousarray(np.asarray(edge_attr, dtype=np.float32))
    W_edge = np.ascontiguousarray(np.asarray(W_edge, dtype=np.float32))
    W1 = np.ascontiguousarray(np.asarray(W1, dtype=np.float32))

    src = np.asarray(edge_index[0], dtype=np.int64)
    nbr = np.asarray(nbr)
    valid = nbr >= 0                                    # [N, K]
    expect = np.arange(E, dtype=np.int64).reshape(N, K)
    assert np.array_equal(np.where(valid, nbr, expect), expect), \
        "kernel assumes nbr[i,k] == i*K+k on valid slots"

    src_slot = src.reshape(N, K)
    xs_slot = x[src_slot]                               # host halo: [N, K, H]
    xs_slot[~valid] = NEG_BIG
    ninv = (~valid).sum(axis=1).astype(np.float32)      # [N]
    ea_slot = edge_attr.reshape(N, K, ED)

    Wbd, BD, ident = _consts(W_edge)
    Wbd16 = Wbd.astype(np_bf16)
    BD16 = BD.astype(np_bf16)
    id16 = ident.astype(np_bf16)

    in_maps = []
    for core in range(NCORES):
        sl = slice(core * NPC, (core + 1) * NPC)
        xae, corr_dev, xres_dev = _stage_core(
            x[sl], xs_slot[sl], ea_slot[sl], ninv[sl])
        in_maps.append({
            "xae": xae, "wbd": Wbd16, "bd": BD16, "id16": id16,
            "id32": ident, "corr": corr_dev, "xres": xres_dev, "w1": W1,
        })
    return in_maps


def kernel(x, edge_index, edge_attr, nbr, W_edge, W1, gamma, beta, W2):
    gamma = np.asarray(gamma, dtype=np.float32)
    beta = np.asarray(beta, dtype=np.float32)
    W2 = np.ascontiguousarray(np.asarray(W2, dtype=np.float32))

    in_maps = build_in_maps(x, edge_index, edge_attr, nbr, W_edge, W1)

    if "p1" not in _compiled:
        _compiled["p1"] = _build_phase1(NT)
        _compiled["p2"] = _build_phase2(NT)

    res1 = run_bass_kernel_spmd(_compiled["p1"], in_maps,
                                core_ids=list(range(NCORES)))

    # host: combine BN stats (tiny 128-vector arithmetic), build scale/shift
    s1 = np.zeros(2 * H, np.float64)
    s2 = np.zeros(2 * H, np.float64)
    for core in range(NCORES):
        st = res1.results[core]["stats"].astype(np.float64)
        s1 += st[:, 0]
        s2 += st[:, 1]
    mean = (s1 / N).astype(np.float32)
    var = (s2 / N).astype(np.float32) - mean * mean
    scale = gamma / np.sqrt(var + 1e-5)
    shift = beta - mean * scale
    ss = np.stack([scale, shift], axis=1).astype(np.float32)  # [128, 2]

    in_maps2 = [{"h1": res1.results[core]["h1"], "ss": ss, "w2": W2}
                for core in range(NCORES)]
    res2 = run_bass_kernel_spmd(_compiled["p2"], in_maps2,
                                core_ids=list(range(NCORES)))

    out = np.empty((N, H), np.float32)
    for core in range(NCORES):
        out[core * NPC:(core + 1) * NPC] = res2.results[core]["out"].T
    return out
